# revision 1
# baseline (speedup 1.0000x reference)
"""FocalLoss + MDCA loss kernel for TRN2, 8-core data-parallel. v4.

reference:
    loss_cls = mean_i[-(1-pt_i) * log(pt_i)],  pt_i = probs[i, targets[i]]
    loss_cal = mean_c |mean_i probs[i,c] - count_c/B|
    out = loss_cls + loss_cal        (GAMMA=1, BETA=1)

Strategy: shard batch (16384) across 8 cores (2048 rows each). Each core:
  - streams its probs shard HBM->SBUF with an inline fp32->fp16 cast (SWDGE)
    as EIGHT [128, 2000] big-tiles: big-tile k covers rows 256k..256k+255
    with partition p holding rows (256k+2p, 256k+2p+1) side by side. Read
    descriptors are 8000 B contiguous (vs 4000 B for one-row tiles): half
    the descriptor count, half the per-packet overhead, and half the load
    on the slow SWDGE engines 7/15.
  - Q7 emission order keeps the baseline's proven shape: 2 big-tile DMAs
    up front, then the const block + the ONE indirect pt gather (its 2048
    tiny descriptors drain while the ring is still shallow), then the
    remaining 6 big-tile DMAs.
  - PE matmul ones[128,1]^T @ probs_fp16 accumulates column sums in PSUM
    (4 x [128,500] per big-tile, same 32 total as before)
  - DVE builds one-hot rows eq[p, j*1000+c] = (c == targets[256k+2p+j]);
    PE matmul ones^T @ eq accumulates the target histogram (exact)
  - pt[p, kj] = probs[256k+2p+j, t] via the indirect gather (exact fp32);
    the focal chain (ACT [pt|ln pt], DVE fused (pt-1)*ln(pt) row-sum, PE
    transpose, ACT accumulate) completes mid-stream
  - tail: last big-tile's 4 colsum matmuls -> PSUM drains split DVE/ACT in
    parallel -> one [1,2001] f32 output DMA
Host combines the 8 cores' colsum/hist/focal partials into the scalar loss
(the gather/unshard step).

The walrus build in this env encodes at most ONE sync wait per instruction;
_split_multi_waits post-processes the scheduled program to hoist extra waits
onto same-engine EventSemaphore carriers.

_compact_sem_ids densely remaps the ~15 semaphores this program touches down
to ids 3..18 and --max-sem-num caps the allocator. (The runtime's end-of-NEFF
sweep still clears all 256 ids — measured fixed cost — but the compact ids
keep the program itself well inside any cap.)
"""

import numpy as np

import concourse.bass as bass
import concourse.bass_utils as _bu
import concourse.mybir as mybir
import concourse.tile as tile
from concourse.bass_utils import run_bass_kernel_spmd

if not getattr(_bu.bir_verify_and_optimise, "_sem_capped", False):
    _orig_bvo = _bu.bir_verify_and_optimise

    def _bvo_capped(*args, **kwargs):
        import concourse.bass_utils as bu

        orig_run = bu.run_command

        def run_with_cap(cmd, **kw):
            if any("codegen" in str(c) for c in cmd):
                cmd = list(cmd) + ["--max-sem-num=32"]
            return orig_run(cmd, **kw)

        bu.run_command = run_with_cap
        try:
            return _orig_bvo(*args, **kwargs)
        finally:
            bu.run_command = orig_run

    _bvo_capped._sem_capped = True
    _bu.bir_verify_and_optimise = _bvo_capped

B, C = 16384, 1000
NCORES = 8
BC = B // NCORES  # 2048 rows per core
P = 128
NB = 8            # big-tiles per core: [128, 2000], 256 rows each
J = 2             # rows per partition per big-tile
W = J * C         # 2000 fp16 columns per big-tile
NT = BC // P      # 16 logical 128-row groups (for targets/pt layout)
CH = 500          # matmul half free-dim (PSUM bank = 512 fp32)
OUT_W = 2001      # [colsum 0:1000 | hist 1000:2000 | focal_sum 2000]
NFRONT = 2        # big-tile DMAs emitted before the Q7 const/gather block

F32 = mybir.dt.float32
F16 = mybir.dt.float16
I16 = mybir.dt.int16
I32 = mybir.dt.int32


def emit_kernel(ctx, tc, probs_d, targ_d, out_d):
    nc = tc.nc
    Alu = mybir.AluOpType
    from concourse.masks import make_identity

    consts = ctx.enter_context(tc.tile_pool(name="consts", bufs=1))
    probs_pool = ctx.enter_context(tc.tile_pool(name="probs_pool", bufs=NB))
    eq_pool = ctx.enter_context(tc.tile_pool(name="eq_pool", bufs=NT))
    psum = ctx.enter_context(tc.tile_pool(name="psum", bufs=1, space="PSUM"))

    # 1) first two big-tile loads start immediately (SDMA drains them while
    # Q7 builds the constants below); partition p of big-tile k reads DRAM
    # rows 256k+2p, 256k+2p+1 — one contiguous 8000 B descriptor.
    def load_tile(k):
        pf16 = probs_pool.tile([P, W], F16, tag="pf16", name=f"pf16_{k}")
        nc.gpsimd.dma_start(
            out=pf16[:],
            in_=probs_d[k * J * P:(k + 1) * J * P, :].rearrange(
                "(p j) c -> p (j c)", p=P, j=J),
        )
        return pf16

    pf16s = [load_tile(k) for k in range(NFRONT)]

    # 2) targets: one [16, 128] contiguous load (HWDGE), PE-transpose to
    # [128, 16] so column i holds targets[128i+p] as per-partition scalars.
    # NOTE: the hist/eq/pt logic below keeps this BASELINE row grouping —
    # the histogram is a multiset count and focal a plain sum, so they
    # don't need to match the big-tile row->partition interleave; only the
    # colsum matmul slices track the new probs layout.
    t_rows_i32 = consts.tile([NT, P], I32, tag="t_rows_i32")
    nc.sync.dma_start(out=t_rows_i32[:], in_=targ_d.rearrange("(i p) -> i p", p=P))

    # 3) constants
    ones = consts.tile([P, 1], F16, tag="ones")
    nc.vector.memset(ones[:], 1.0)
    iota_i16 = consts.tile([P, C], I16, tag="iota_i16")
    nc.gpsimd.iota(iota_i16[:], pattern=[[1, C]], base=0, channel_multiplier=0)
    iota_f16 = consts.tile([P, C], F16, tag="iota_f16")
    nc.vector.tensor_copy(iota_f16[:], iota_i16[:])
    identity = consts.tile([P, P], F32, tag="identity")
    make_identity(nc, identity[:])

    t_rows_f32 = consts.tile([NT, P], F32, tag="t_rows_f32")
    # gpsimd (not DVE) so the PE transpose below has single-engine producers
    nc.gpsimd.tensor_copy(t_rows_f32[:], t_rows_i32[:])
    t_ps = psum.tile([P, NT], F32, tag="t_ps")
    nc.tensor.transpose(t_ps[:], t_rows_f32[:], identity[:NT, :NT])
    t_cols = consts.tile([P, NT], F32, tag="t_cols")
    nc.vector.tensor_copy(t_cols[:], t_ps[:])
    t_cols_i32 = consts.tile([P, NT], I32, tag="t_cols_i32")
    nc.vector.tensor_copy(t_cols_i32[:], t_ps[:])

    # pt[p, i] = probs[128i + p, t] in ONE indirect gather (exact fp32),
    # emitted before the bulk probs loads so its 2048 descriptors drain on a
    # near-quiet ring (the ring is 8x the default size).
    rows_i32 = consts.tile([P, NT], I32, tag="rows_i32")
    nc.gpsimd.iota(rows_i32[:], pattern=[[P, NT]], base=0, channel_multiplier=1)
    offs = consts.tile([P, NT], I32, tag="offs")
    nc.vector.tensor_scalar(out=offs[:], in0=rows_i32[:], scalar1=float(C),
                            scalar2=None, op0=Alu.mult)
    nc.vector.tensor_tensor(out=offs[:], in0=offs[:], in1=t_cols_i32[:],
                            op=Alu.add)
    pt_all = consts.tile([P, NT], F32, tag="pt_all")
    nc.gpsimd.indirect_dma_start(
        out=pt_all[:], out_offset=None,
        in_=probs_d.rearrange("a b -> (a b)")[:, None],
        in_offset=bass.IndirectOffsetOnAxis(ap=offs[:], axis=0),
    )

    # remaining big-tile loads
    pf16s += [load_tile(k) for k in range(NFRONT, NB)]

    # persistent accumulators
    cs_ps = [psum.tile([1, CH], F32, tag=f"cs_ps{h}", name=f"cs_ps{h}")
             for h in range(2)]
    hs_ps = [psum.tile([1, CH], F32, tag=f"hs_ps{h}", name=f"hs_ps{h}")
             for h in range(2)]

    # 4a) one-hot rows eq_i[p, c] = (c == targets[128i+p]) — baseline row
    # grouping, DVE-paced while the probs DMAs stream in.
    eqs = []
    for i in range(NT):
        eq = eq_pool.tile([P, C], F16, tag="eq", name=f"eq_{i}")
        nc.vector.tensor_scalar(
            out=eq[:], in0=iota_f16[:], scalar1=t_cols[:, i:i + 1], scalar2=None,
            op0=Alu.is_equal,
        )
        eqs.append(eq)

    # 4b) all histogram matmuls as one dense DMA-independent block: early
    # back-to-back PE work warms the HAM clock gate (2.4 GHz) before the
    # DMA-paced colsum matmuls arrive.
    for i in range(NT):
        first, last = (i == 0), (i == NT - 1)
        for h in range(2):
            sl = slice(h * CH, (h + 1) * CH)
            nc.tensor.matmul(hs_ps[h][:], ones[:], eqs[i][:, sl],
                             start=first, stop=last)

    # 4c) DMA-paced colsum matmuls: 4 x [128,500] per big-tile, banks
    # alternating so each bank accumulates 16 matmuls.
    for k in range(NB):
        for q in range(2 * J):
            sl = slice(q * CH, (q + 1) * CH)
            nc.tensor.matmul(cs_ps[q % 2][:], ones[:], pf16s[k][:, sl],
                             start=(k == 0 and q < 2),
                             stop=(k == NB - 1 and q >= 2 * J - 2))

    # 5) focal tail: focal[p] = sum_kj (pt - 1) * ln(pt).
    # Stage [pt | ln(pt)] side by side via ACT so the DVE reduce depends on a
    # single engine.
    pl = consts.tile([P, 2 * NT], F32, tag="pl")
    nc.scalar.copy(pl[:, 0:NT], pt_all[:])
    nc.scalar.activation(pl[:, NT:2 * NT], pt_all[:],
                         mybir.ActivationFunctionType.Ln)
    junk = consts.tile([P, NT], F32, tag="junk")
    focal = consts.tile([P, 1], F32, tag="focal")
    nc.vector.scalar_tensor_tensor(
        out=junk[:], in0=pl[:, 0:NT], scalar=1.0, in1=pl[:, NT:2 * NT],
        op0=Alu.subtract, op1=Alu.mult, accum_out=focal[:],
    )
    # reduce focal over partitions: PE transpose to a row, ACT accumulates
    fc_t = psum.tile([1, P], F32, tag="fc_t")
    nc.tensor.transpose(fc_t[:], focal[:], identity[:])

    # 6) pack [colsum | hist | focal_sum] into one row, single output DMA.
    # hist halves drain on ACT mid-stream; the colsum halves drain in
    # parallel right after the last matmul (DVE half 0, ACT half 1).
    out_sb = consts.tile([1, OUT_W], F32, tag="out_sb")
    for h in range(2):
        nc.scalar.copy(out_sb[:, C + h * CH:C + (h + 1) * CH], hs_ps[h][:])
    fc_row = consts.tile([1, P], F32, tag="fc_row")
    nc.scalar.activation(fc_row[:], fc_t[:],
                         mybir.ActivationFunctionType.Copy,
                         accum_out=out_sb[:, 2 * C:2 * C + 1])
    nc.vector.tensor_copy(out_sb[:, 0:CH], cs_ps[0][:])
    nc.scalar.copy(out_sb[:, CH:2 * CH], cs_ps[1][:])
    nc.sync.dma_start(out=out_d[:, :], in_=out_sb[:])


def _split_multi_waits(nc):
    """The walrus build in this env encodes at most ONE sync wait per
    instruction (newer Tile emits several, e.g. on its tail drain). Hoist
    extra waits onto EventSemaphore carrier instructions inserted just
    before, on the same engine — same-engine program order makes this
    semantically identical."""
    n = 0
    for f in nc.m.functions:
        for blk in f.blocks:
            il = blk.instructions
            i = 0
            while i < len(il):
                inst = il[i]
                si = inst.sync_info
                ws = list(si.on_wait) if si is not None else []
                if len(ws) > 1:
                    for w in ws[:-1]:
                        ev = mybir.InstEventSemaphore(
                            name=f"I-waitsplit-{n}", ins=[], outs=[])
                        n += 1
                        ev.engine = inst.engine
                        ev.sync_info = mybir.SyncInfo(on_wait=[w], on_update=[])
                        il.insert(i, ev)
                        i += 1
                    inst.sync_info = mybir.SyncInfo(
                        on_wait=[ws[-1]], on_update=list(si.on_update))
                i += 1


def _compact_sem_ids(nc, base=3):
    """Tile/bass allocate semaphore ids from ~151 up; remap every semaphore
    this program touches down to [base, base+n) so the program sits inside
    a small --max-sem-num cap. ids 0-2 stay free for the compiler's own
    barriers."""
    def insts():
        for f in nc.m.functions:
            for b in f.blocks:
                yield from b.instructions

    used = set()
    for inst in insts():
        si = inst.sync_info
        if si:
            for w in list(si.on_wait):
                if w.sync_type == "semaphore":
                    used.add(w.id)
            for u in list(si.on_update):
                if u.sync_type == "semaphore":
                    used.add(u.id)
    m = {old: base + i for i, old in enumerate(sorted(used))}
    for inst in insts():
        si = inst.sync_info
        if si:
            ws, us = list(si.on_wait), list(si.on_update)
            changed = False
            for w in ws:
                if w.sync_type == "semaphore" and w.id in m:
                    w.id = m[w.id]
                    changed = True
            for u in us:
                if u.sync_type == "semaphore" and u.id in m:
                    u.id = m[u.id]
                    changed = True
            if changed:
                inst.sync_info = mybir.SyncInfo(on_wait=ws, on_update=us)
        if (type(inst).__name__ == "InstISA"
                and getattr(inst, "op_name", "") == "EVENT_SEMAPHORE_RANGE_CLEAR"):
            d = inst.ant_dict
            ids = [m[x] for x in range(d["range_first"], d["range_last"] + 1)
                   if x in m]
            nf, nl = (min(ids), max(ids)) if ids else (base, base)
            d["range_first"], d["range_last"] = nf, nl
            v = list(inst.instr)
            v[13], v[14] = nf, nl
            inst.instr = v
            inst.ant_dict = d


_cached_nc = {}


def build_nc(split_waits=True):
    global _cached_nc
    if split_waits in _cached_nc:
        return _cached_nc[split_waits]
    from contextlib import ExitStack

    nc = bass.Bass("TRN2", dynamic_dma_scratch_size=131072)
    probs_d = nc.dram_tensor("probs", [BC, C], F32, kind="ExternalInput").ap()
    targ_d = nc.dram_tensor("targets", [BC], I32, kind="ExternalInput").ap()
    out_d = nc.dram_tensor("out_all", [1, OUT_W], F32, kind="ExternalOutput").ap()

    with tile.TileContext(nc) as tc:
        with ExitStack() as ctx:
            emit_kernel(ctx, tc, probs_d, targ_d, out_d)
    if split_waits:
        _split_multi_waits(nc)
    _compact_sem_ids(nc)
    _cached_nc[split_waits] = nc
    return nc


def make_in_maps(probs, targets):
    probs = np.ascontiguousarray(np.asarray(probs), dtype=np.float32)
    targets = np.asarray(targets).astype(np.int32)
    assert probs.shape == (B, C) and targets.shape == (B,)
    return [
        {
            "probs": probs[k * BC:(k + 1) * BC],
            "targets": np.ascontiguousarray(targets[k * BC:(k + 1) * BC]),
        }
        for k in range(NCORES)
    ]


def combine(results):
    cs = np.zeros(C, np.float64)
    hs = np.zeros(C, np.float64)
    fc = 0.0
    for r in results:
        row = r["out_all"].reshape(OUT_W).astype(np.float64)
        cs += row[0:C]
        hs += row[C:2 * C]
        fc += row[2 * C]
    loss_cls = fc / B
    loss_cal = float(np.mean(np.abs(cs / B - hs / B)))
    return np.asarray(loss_cls + 1.0 * loss_cal, dtype=np.float32)


def run_spmd(probs, targets, **kwargs):
    nc = build_nc()
    in_maps = make_in_maps(probs, targets)
    return run_bass_kernel_spmd(nc, in_maps, list(range(NCORES)), **kwargs)


def kernel(probs, targets):
    res = run_spmd(probs, targets)
    return combine(res.results)



# revision 2
# speedup vs baseline: 1.0794x; 1.0794x over previous
"""FocalLoss + MDCA loss kernel for TRN2, 8-core data-parallel. v4.

reference:
    loss_cls = mean_i[-(1-pt_i) * log(pt_i)],  pt_i = probs[i, targets[i]]
    loss_cal = mean_c |mean_i probs[i,c] - count_c/B|
    out = loss_cls + loss_cal        (GAMMA=1, BETA=1)

Strategy: shard batch (16384) across 8 cores (2048 rows each). Each core:
  - streams its probs shard HBM->SBUF with an inline fp32->fp16 cast (SWDGE)
    as EIGHT [128, 2000] big-tiles: big-tile k covers rows 256k..256k+255
    with partition p holding rows (256k+2p, 256k+2p+1) side by side. Read
    descriptors are 8000 B contiguous (vs 4000 B for one-row tiles): half
    the descriptor count, half the per-packet overhead, and half the load
    on the slow SWDGE engines 7/15.
  - Q7 emission order keeps the baseline's proven shape: 2 big-tile DMAs
    up front, then the const block + the ONE indirect pt gather (its 2048
    tiny descriptors drain while the ring is still shallow), then the
    remaining 6 big-tile DMAs.
  - PE matmul ones[128,1]^T @ probs_fp16 accumulates column sums in PSUM
    (4 x [128,500] per big-tile, same 32 total as before)
  - DVE builds one-hot rows eq[p, j*1000+c] = (c == targets[256k+2p+j]);
    PE matmul ones^T @ eq accumulates the target histogram (exact)
  - pt[p, kj] = probs[256k+2p+j, t] via the indirect gather (exact fp32);
    the focal chain (ACT [pt|ln pt], DVE fused (pt-1)*ln(pt) row-sum, PE
    transpose, ACT accumulate) completes mid-stream
  - tail: last big-tile's 4 colsum matmuls -> PSUM drains split DVE/ACT in
    parallel -> one [1,2001] f32 output DMA
Host combines the 8 cores' colsum/hist/focal partials into the scalar loss
(the gather/unshard step).

The walrus build in this env encodes at most ONE sync wait per instruction;
_split_multi_waits post-processes the scheduled program to hoist extra waits
onto same-engine EventSemaphore carriers.

_compact_sem_ids densely remaps the ~15 semaphores this program touches down
to ids 3..18 and --max-sem-num caps the allocator. (The runtime's end-of-NEFF
sweep still clears all 256 ids — measured fixed cost — but the compact ids
keep the program itself well inside any cap.)
"""

import numpy as np

import concourse.bass as bass
import concourse.bass_utils as _bu
import concourse.mybir as mybir
import concourse.tile as tile
from concourse.bass_utils import run_bass_kernel_spmd

if not getattr(_bu.bir_verify_and_optimise, "_sem_capped", False):
    _orig_bvo = _bu.bir_verify_and_optimise

    def _bvo_capped(*args, **kwargs):
        import concourse.bass_utils as bu

        orig_run = bu.run_command

        def run_with_cap(cmd, **kw):
            if any("codegen" in str(c) for c in cmd):
                cmd = list(cmd) + ["--max-sem-num=32"]
                import os as _os
                extra = _os.environ.get("KERNEL_WALRUS_EXTRA", "")
                if extra:
                    cmd = cmd + extra.split()
            return orig_run(cmd, **kw)

        bu.run_command = run_with_cap
        try:
            return _orig_bvo(*args, **kwargs)
        finally:
            bu.run_command = orig_run

    _bvo_capped._sem_capped = True
    _bu.bir_verify_and_optimise = _bvo_capped

B, C = 16384, 1000
NCORES = 8
BC = B // NCORES  # 2048 rows per core
P = 128
NB = 8            # big-tiles per core: [128, 2000], 256 rows each
J = 2             # rows per partition per big-tile
W = J * C         # 2000 fp16 columns per big-tile
NT = BC // P      # 16 logical 128-row groups (for targets/pt layout)
CH = 500          # matmul half free-dim (PSUM bank = 512 fp32)
OUT_W = 2001      # [colsum 0:1000 | hist 1000:2000 | focal_sum 2000]
NFRONT = 2        # big-tile DMAs emitted before the Q7 const/gather block

F32 = mybir.dt.float32
F16 = mybir.dt.float16
I16 = mybir.dt.int16
I32 = mybir.dt.int32


def emit_kernel(ctx, tc, probs_d, targ_d, out_d):
    nc = tc.nc
    Alu = mybir.AluOpType
    from concourse.masks import make_identity

    consts = ctx.enter_context(tc.tile_pool(name="consts", bufs=1))
    probs_pool = ctx.enter_context(tc.tile_pool(name="probs_pool", bufs=NB))
    eq_pool = ctx.enter_context(tc.tile_pool(name="eq_pool", bufs=NT))
    psum = ctx.enter_context(tc.tile_pool(name="psum", bufs=1, space="PSUM"))

    # 1) first two big-tile loads start immediately (SDMA drains them while
    # Q7 builds the constants below); partition p of big-tile k reads DRAM
    # rows 256k+2p, 256k+2p+1 — one contiguous 8000 B descriptor.
    def load_tile(k):
        pf16 = probs_pool.tile([P, W], F16, tag="pf16", name=f"pf16_{k}")
        nc.gpsimd.dma_start(
            out=pf16[:],
            in_=probs_d[k * J * P:(k + 1) * J * P, :].rearrange(
                "(p j) c -> p (j c)", p=P, j=J),
        )
        return pf16

    pf16s = [load_tile(k) for k in range(NFRONT)]

    # 2) targets: one [16, 128] contiguous load (HWDGE), PE-transpose to
    # [128, 16] so column i holds targets[128i+p] as per-partition scalars.
    # NOTE: the hist/eq/pt logic below keeps this BASELINE row grouping —
    # the histogram is a multiset count and focal a plain sum, so they
    # don't need to match the big-tile row->partition interleave; only the
    # colsum matmul slices track the new probs layout.
    t_rows_i32 = consts.tile([NT, P], I32, tag="t_rows_i32")
    nc.sync.dma_start(out=t_rows_i32[:], in_=targ_d.rearrange("(i p) -> i p", p=P))

    # 3) constants
    ones = consts.tile([P, 1], F16, tag="ones")
    nc.vector.memset(ones[:], 1.0)
    iota_i16 = consts.tile([P, C], I16, tag="iota_i16")
    nc.gpsimd.iota(iota_i16[:], pattern=[[1, C]], base=0, channel_multiplier=0)
    iota_f16 = consts.tile([P, C], F16, tag="iota_f16")
    nc.vector.tensor_copy(iota_f16[:], iota_i16[:])
    identity = consts.tile([P, P], F32, tag="identity")
    make_identity(nc, identity[:])

    t_rows_f32 = consts.tile([NT, P], F32, tag="t_rows_f32")
    # gpsimd (not DVE) so the PE transpose below has single-engine producers
    nc.gpsimd.tensor_copy(t_rows_f32[:], t_rows_i32[:])
    t_ps = psum.tile([P, NT], F32, tag="t_ps")
    nc.tensor.transpose(t_ps[:], t_rows_f32[:], identity[:NT, :NT])
    t_cols = consts.tile([P, NT], F32, tag="t_cols")
    nc.vector.tensor_copy(t_cols[:], t_ps[:])
    t_cols_i32 = consts.tile([P, NT], I32, tag="t_cols_i32")
    nc.vector.tensor_copy(t_cols_i32[:], t_ps[:])

    # pt[p, i] = probs[128i + p, t] in ONE indirect gather (exact fp32),
    # emitted before the bulk probs loads so its 2048 descriptors drain on a
    # near-quiet ring (the ring is 8x the default size).
    rows_i32 = consts.tile([P, NT], I32, tag="rows_i32")
    nc.gpsimd.iota(rows_i32[:], pattern=[[P, NT]], base=0, channel_multiplier=1)
    offs = consts.tile([P, NT], I32, tag="offs")
    nc.vector.tensor_scalar(out=offs[:], in0=rows_i32[:], scalar1=float(C),
                            scalar2=None, op0=Alu.mult)
    nc.vector.tensor_tensor(out=offs[:], in0=offs[:], in1=t_cols_i32[:],
                            op=Alu.add)
    pt_all = consts.tile([P, NT], F32, tag="pt_all")
    nc.gpsimd.indirect_dma_start(
        out=pt_all[:], out_offset=None,
        in_=probs_d.rearrange("a b -> (a b)")[:, None],
        in_offset=bass.IndirectOffsetOnAxis(ap=offs[:], axis=0),
    )

    # remaining big-tile loads
    pf16s += [load_tile(k) for k in range(NFRONT, NB)]

    # persistent accumulators
    cs_ps = [psum.tile([1, CH], F32, tag=f"cs_ps{h}", name=f"cs_ps{h}")
             for h in range(2)]
    hs_ps = [psum.tile([1, CH], F32, tag=f"hs_ps{h}", name=f"hs_ps{h}")
             for h in range(2)]

    # 4a) one-hot rows eq_i[p, c] = (c == targets[128i+p]) — baseline row
    # grouping, DVE-paced while the probs DMAs stream in.
    eqs = []
    for i in range(NT):
        eq = eq_pool.tile([P, C], F16, tag="eq", name=f"eq_{i}")
        nc.vector.tensor_scalar(
            out=eq[:], in0=iota_f16[:], scalar1=t_cols[:, i:i + 1], scalar2=None,
            op0=Alu.is_equal,
        )
        eqs.append(eq)

    # 4b) all histogram matmuls as one dense DMA-independent block: early
    # back-to-back PE work warms the HAM clock gate (2.4 GHz) before the
    # DMA-paced colsum matmuls arrive.
    for i in range(NT):
        first, last = (i == 0), (i == NT - 1)
        for h in range(2):
            sl = slice(h * CH, (h + 1) * CH)
            nc.tensor.matmul(hs_ps[h][:], ones[:], eqs[i][:, sl],
                             start=first, stop=last)

    # 4c) DMA-paced colsum matmuls: 4 x [128,500] per big-tile, banks
    # alternating so each bank accumulates 16 matmuls.
    for k in range(NB):
        for q in range(2 * J):
            sl = slice(q * CH, (q + 1) * CH)
            nc.tensor.matmul(cs_ps[q % 2][:], ones[:], pf16s[k][:, sl],
                             start=(k == 0 and q < 2),
                             stop=(k == NB - 1 and q >= 2 * J - 2))

    # 5) focal tail: focal[p] = sum_kj (pt - 1) * ln(pt).
    # Stage [pt | ln(pt)] side by side via ACT so the DVE reduce depends on a
    # single engine.
    pl = consts.tile([P, 2 * NT], F32, tag="pl")
    nc.scalar.copy(pl[:, 0:NT], pt_all[:])
    nc.scalar.activation(pl[:, NT:2 * NT], pt_all[:],
                         mybir.ActivationFunctionType.Ln)
    junk = consts.tile([P, NT], F32, tag="junk")
    focal = consts.tile([P, 1], F32, tag="focal")
    nc.vector.scalar_tensor_tensor(
        out=junk[:], in0=pl[:, 0:NT], scalar=1.0, in1=pl[:, NT:2 * NT],
        op0=Alu.subtract, op1=Alu.mult, accum_out=focal[:],
    )
    # reduce focal over partitions: PE transpose to a row, ACT accumulates
    fc_t = psum.tile([1, P], F32, tag="fc_t")
    nc.tensor.transpose(fc_t[:], focal[:], identity[:])

    # 6) pack [colsum | hist | focal_sum] into one row, single output DMA.
    # hist halves drain on ACT mid-stream; the colsum halves drain in
    # parallel right after the last matmul (DVE half 0, ACT half 1).
    out_sb = consts.tile([1, OUT_W], F32, tag="out_sb")
    for h in range(2):
        nc.scalar.copy(out_sb[:, C + h * CH:C + (h + 1) * CH], hs_ps[h][:])
    fc_row = consts.tile([1, P], F32, tag="fc_row")
    nc.scalar.activation(fc_row[:], fc_t[:],
                         mybir.ActivationFunctionType.Copy,
                         accum_out=out_sb[:, 2 * C:2 * C + 1])
    nc.vector.tensor_copy(out_sb[:, 0:CH], cs_ps[0][:])
    nc.scalar.copy(out_sb[:, CH:2 * CH], cs_ps[1][:])
    nc.sync.dma_start(out=out_d[:, :], in_=out_sb[:])


def _split_multi_waits(nc):
    """The walrus build in this env encodes at most ONE sync wait per
    instruction (newer Tile emits several, e.g. on its tail drain). Hoist
    extra waits onto EventSemaphore carrier instructions inserted just
    before, on the same engine — same-engine program order makes this
    semantically identical."""
    n = 0
    for f in nc.m.functions:
        for blk in f.blocks:
            il = blk.instructions
            i = 0
            while i < len(il):
                inst = il[i]
                si = inst.sync_info
                ws = list(si.on_wait) if si is not None else []
                if len(ws) > 1:
                    for w in ws[:-1]:
                        ev = mybir.InstEventSemaphore(
                            name=f"I-waitsplit-{n}", ins=[], outs=[])
                        n += 1
                        ev.engine = inst.engine
                        ev.sync_info = mybir.SyncInfo(on_wait=[w], on_update=[])
                        il.insert(i, ev)
                        i += 1
                    inst.sync_info = mybir.SyncInfo(
                        on_wait=[ws[-1]], on_update=list(si.on_update))
                i += 1


def _compact_sem_ids(nc, base=3):
    """Tile/bass allocate semaphore ids from ~151 up; remap every semaphore
    this program touches down to [base, base+n) so the program sits inside
    a small --max-sem-num cap. ids 0-2 stay free for the compiler's own
    barriers."""
    def insts():
        for f in nc.m.functions:
            for b in f.blocks:
                yield from b.instructions

    used = set()
    for inst in insts():
        si = inst.sync_info
        if si:
            for w in list(si.on_wait):
                if w.sync_type == "semaphore":
                    used.add(w.id)
            for u in list(si.on_update):
                if u.sync_type == "semaphore":
                    used.add(u.id)
    m = {old: base + i for i, old in enumerate(sorted(used))}
    for inst in insts():
        si = inst.sync_info
        if si:
            ws, us = list(si.on_wait), list(si.on_update)
            changed = False
            for w in ws:
                if w.sync_type == "semaphore" and w.id in m:
                    w.id = m[w.id]
                    changed = True
            for u in us:
                if u.sync_type == "semaphore" and u.id in m:
                    u.id = m[u.id]
                    changed = True
            if changed:
                inst.sync_info = mybir.SyncInfo(on_wait=ws, on_update=us)
        if (type(inst).__name__ == "InstISA"
                and getattr(inst, "op_name", "") == "EVENT_SEMAPHORE_RANGE_CLEAR"):
            d = inst.ant_dict
            ids = [m[x] for x in range(d["range_first"], d["range_last"] + 1)
                   if x in m]
            nf, nl = (min(ids), max(ids)) if ids else (base, base)
            d["range_first"], d["range_last"] = nf, nl
            v = list(inst.instr)
            v[13], v[14] = nf, nl
            inst.instr = v
            inst.ant_dict = d


_cached_nc = {}


def build_nc(split_waits=True):
    global _cached_nc
    if split_waits in _cached_nc:
        return _cached_nc[split_waits]
    from contextlib import ExitStack

    nc = bass.Bass("TRN2", dynamic_dma_scratch_size=131072)
    probs_d = nc.dram_tensor("probs", [BC, C], F32, kind="ExternalInput").ap()
    targ_d = nc.dram_tensor("targets", [BC], I32, kind="ExternalInput").ap()
    out_d = nc.dram_tensor("out_all", [1, OUT_W], F32, kind="ExternalOutput").ap()

    with tile.TileContext(nc) as tc:
        with ExitStack() as ctx:
            emit_kernel(ctx, tc, probs_d, targ_d, out_d)
    if split_waits:
        _split_multi_waits(nc)
    _compact_sem_ids(nc)
    _cached_nc[split_waits] = nc
    return nc


def make_in_maps(probs, targets):
    probs = np.ascontiguousarray(np.asarray(probs), dtype=np.float32)
    targets = np.asarray(targets).astype(np.int32)
    assert probs.shape == (B, C) and targets.shape == (B,)
    return [
        {
            "probs": probs[k * BC:(k + 1) * BC],
            "targets": np.ascontiguousarray(targets[k * BC:(k + 1) * BC]),
        }
        for k in range(NCORES)
    ]


def combine(results):
    cs = np.zeros(C, np.float64)
    hs = np.zeros(C, np.float64)
    fc = 0.0
    for r in results:
        row = r["out_all"].reshape(OUT_W).astype(np.float64)
        cs += row[0:C]
        hs += row[C:2 * C]
        fc += row[2 * C]
    loss_cls = fc / B
    loss_cal = float(np.mean(np.abs(cs / B - hs / B)))
    return np.asarray(loss_cls + 1.0 * loss_cal, dtype=np.float32)


def run_spmd(probs, targets, **kwargs):
    nc = build_nc()
    in_maps = make_in_maps(probs, targets)
    return run_bass_kernel_spmd(nc, in_maps, list(range(NCORES)), **kwargs)


def kernel(probs, targets):
    res = run_spmd(probs, targets)
    return combine(res.results)

